# revision 1
# baseline (speedup 1.0000x reference)
"""FastGTLayer GNN message passing on 8 Trainium2 NeuronCores.

Strategy (destination-sharded, gather + selection-matmul scatter):
- Host: softmax(weight) -> per-edge per-channel weights w_c = filt[c,t]*ev[t,e].
  Edges sharded by destination row range (6250 rows/core), sorted by row,
  grouped into 32-row "blocks"; each block padded to a per-block tile quota
  (max over cores) of 128-edge tiles.
- Device (SPMD, one program on 8 cores): for each chunk of 8 blocks,
  dma_gather fetches H rows (both channels interleaved, 512B/edge) by int16
  index with a biased base (in_=H_pre[17232:], idx = col-17232 in [-17232,32767]);
  DVE builds weighted one-hot selection matrices sel_c[e,r] = w_c[e]*(row==r);
  PE scatter-adds via matmul psum_c[64f,32r] += g_c^T @ sel_c over the block's
  tiles; ACT evicts psum to SBUF; HWDGE writes [128,(c,d)] x rows to HBM.
- Host: transpose per-core [128, rows] outputs into [C, N, D].
"""
import sys
if "/opt/trn_rl_repo" not in sys.path:
    sys.path.insert(0, "/opt/trn_rl_repo")

import numpy as np

C, T, N, E, D = 2, 4, 50000, 400000, 64
M = T * E
NCORES = 8
RPC = N // NCORES          # 6250 destination rows per core
R = 32                     # rows per block (psum window)
NBLOCKS = (RPC + R - 1) // R   # 196 (last block partially used)
BIAS = N - 32768           # 17232; idx = col - BIAS in [-17232, 32767]
PADCOL = 40000             # pad slots gather this row (positive idx), weight 0
CHUNK_BLOCKS = 8           # blocks per dma_gather instruction

_prog_cache = {}


def _build_program(quotas, tt, skip_gather=False, skip_compute=False,
                   nqueues=1, chunk_blocks=None, gbufs=2, selbufs=2, pbufs=2,
                   scratch=16384, repeat=1):
    """Build the SPMD Bass program for per-block tile quotas `quotas` (len
    NBLOCKS, sum tt). Returns compiled Bacc instance."""
    from concourse import bacc, mybir
    import concourse.tile as tile
    from concourse.bass import AP

    nc = bacc.Bacc("TRN2", num_swdge_queues=nqueues, dynamic_dma_scratch_size=scratch)
    hpre = nc.dram_tensor("hpre", [N, 2 * D], mybir.dt.float32, kind="ExternalInput")
    idx = nc.dram_tensor("idx", [128, tt * 8], mybir.dt.int16, kind="ExternalInput")
    rowl = nc.dram_tensor("rowl", [128, tt], mybir.dt.float32, kind="ExternalInput")
    w0 = nc.dram_tensor("w0", [128, tt], mybir.dt.float32, kind="ExternalInput")
    w1 = nc.dram_tensor("w1", [128, tt], mybir.dt.float32, kind="ExternalInput")
    iota = nc.dram_tensor("iota", [128, R], mybir.dt.float32, kind="ExternalInput")
    out_local = nc.dram_tensor("out_local", [128, NBLOCKS * R], mybir.dt.float32,
                               kind="ExternalOutput")

    cb_n = chunk_blocks or CHUNK_BLOCKS
    nchunks = (NBLOCKS + cb_n - 1) // cb_n
    tile_base = np.concatenate([[0], np.cumsum(quotas)]).astype(int)

    with tile.TileContext(nc) as tc:
        with tc.tile_pool(name="meta", bufs=1) as mp, \
             tc.tile_pool(name="gp", bufs=gbufs) as gp, \
             tc.tile_pool(name="selp", bufs=selbufs) as selp, \
             tc.tile_pool(name="stp", bufs=2) as stp, \
             tc.tile_pool(name="pp", bufs=pbufs, space="PSUM") as pp:
            idx_t = mp.tile([128, tt * 8], mybir.dt.int16)
            rowl_t = mp.tile([128, tt], mybir.dt.float32)
            w0_t = mp.tile([128, tt], mybir.dt.float32)
            w1_t = mp.tile([128, tt], mybir.dt.float32)
            iota_t = mp.tile([128, R], mybir.dt.float32)

            nc.gpsimd.dma_start(out=idx_t[:], in_=idx[:])
            nc.gpsimd.dma_start(out=rowl_t[:], in_=rowl[:])
            nc.gpsimd.dma_start(out=w0_t[:], in_=w0[:])
            nc.gpsimd.dma_start(out=w1_t[:], in_=w1[:])
            nc.gpsimd.dma_start(out=iota_t[:], in_=iota[:])

            iota_ap = iota_t[:]

            for rep in range(repeat):
              for c in range(nchunks):
                b0 = c * cb_n
                b1 = min(b0 + cb_n, NBLOCKS)
                tb0, tb1 = tile_base[b0], tile_base[b1]
                ct = int(tb1 - tb0)          # tiles in this chunk
                nidx = ct * 128

                g_t = gp.tile([128, ct, 2 * D], mybir.dt.float32, tag="g")
                if not skip_gather:
                    nc.gpsimd.dma_gather(
                        g_t[:],
                        hpre[BIAS:, :],
                        idx_t[:, tb0 * 8: tb1 * 8],
                        nidx,
                        nidx,
                        2 * D,
                        queue_num=(c % nqueues),
                        single_packet=False,
                    )

                stage = stp.tile([128, (b1 - b0) * R], mybir.dt.float32, tag="st")
                for b in [] if skip_compute else range(b0, b1):
                    kb = int(quotas[b])
                    t0 = int(tile_base[b])      # global tile index
                    lt0 = t0 - tb0              # tile index within chunk
                    iota_b = AP(iota_ap.tensor, iota_ap.offset,
                                [iota_ap.ap[0], [0, kb], iota_ap.ap[1]])
                    sel_eq = selp.tile([128, kb, R], mybir.dt.float32, tag="se")
                    sel0 = selp.tile([128, kb, R], mybir.dt.float32, tag="s0")
                    sel1 = selp.tile([128, kb, R], mybir.dt.float32, tag="s1")
                    nc.vector.tensor_tensor(
                        out=sel_eq[:],
                        in0=rowl_t[:, t0:t0 + kb].to_broadcast([128, kb, R]),
                        in1=iota_b, op=mybir.AluOpType.is_equal)
                    nc.vector.tensor_tensor(
                        out=sel0[:], in0=sel_eq[:],
                        in1=w0_t[:, t0:t0 + kb].to_broadcast([128, kb, R]),
                        op=mybir.AluOpType.mult)
                    nc.vector.tensor_tensor(
                        out=sel1[:], in0=sel_eq[:],
                        in1=w1_t[:, t0:t0 + kb].to_broadcast([128, kb, R]),
                        op=mybir.AluOpType.mult)

                    ps0 = pp.tile([64, R], mybir.dt.float32, space="PSUM", tag="p0")
                    ps1 = pp.tile([64, R], mybir.dt.float32, space="PSUM", tag="p1")
                    for k in range(kb):
                        nc.tensor.matmul(out=ps0[:], lhsT=g_t[:, lt0 + k, 0:D],
                                         rhs=sel0[:, k, :],
                                         start=(k == 0), stop=(k == kb - 1))
                        nc.tensor.matmul(out=ps1[:], lhsT=g_t[:, lt0 + k, D:2 * D],
                                         rhs=sel1[:, k, :],
                                         start=(k == 0), stop=(k == kb - 1))
                    so = (b - b0) * R
                    nc.scalar.copy(out=stage[0:64, so:so + R], in_=ps0[:])
                    nc.scalar.copy(out=stage[64:128, so:so + R], in_=ps1[:])

                if skip_compute:
                    nc.vector.memset(stage[:], 0.0)
                nc.sync.dma_start(out=out_local[:, b0 * R: b1 * R], in_=stage[:])

    nc.compile()
    return nc


def _prepare(H_, edge_index, edge_values, weight):
    """Host-side preprocessing. Returns (quotas, tt, in_maps)."""
    H_ = np.asarray(H_, dtype=np.float32)
    edge_index = np.asarray(edge_index)
    edge_values = np.asarray(edge_values, dtype=np.float32)
    weight = np.asarray(weight, dtype=np.float64)

    # softmax over edge types per channel
    wexp = np.exp(weight - weight.max(axis=1, keepdims=True))
    filt = (wexp / wexp.sum(axis=1, keepdims=True)).astype(np.float32)  # [C,T]

    row = np.ascontiguousarray(edge_index[:, 0, :]).reshape(-1).astype(np.int64)
    col = np.ascontiguousarray(edge_index[:, 1, :]).reshape(-1).astype(np.int64)
    ev = edge_values.reshape(-1)
    tt_of_edge = np.repeat(np.arange(T), E)
    wc = filt[:, tt_of_edge] * ev[None, :]      # [C, M]

    H_pre = np.ascontiguousarray(np.transpose(H_, (1, 0, 2)).reshape(N, C * D))

    core = row // RPC
    row_local = row - core * RPC
    block = row_local // R                       # [0, NBLOCKS)
    # global sort groups edges by (core, block) since both are row-range based
    perm = np.argsort(row, kind="stable")
    core_s = core[perm]
    block_s = block[perm]
    col_s = col[perm]
    rl_s = (row_local[perm] % R).astype(np.float32)
    w0_s = wc[0][perm]
    w1_s = wc[1][perm]

    cb = core_s * NBLOCKS + block_s
    counts = np.bincount(cb, minlength=NCORES * NBLOCKS).reshape(NCORES, NBLOCKS)
    quotas = np.ceil(counts.max(axis=0) / 128).astype(np.int64)
    quotas = np.maximum(quotas, 1)
    tt = int(quotas.sum())
    tile_base = np.concatenate([[0], np.cumsum(quotas)]).astype(np.int64)

    # slot index for each sorted edge: block b of core k occupies slots
    # [tile_base[b]*128, +counts[k,b]) in core k's slot space
    # offsets within (core, block) groups:
    group_starts = np.concatenate([[0], np.cumsum(counts.reshape(-1))])[:-1]
    within = np.arange(M) - group_starts[cb]     # position within group
    slot = tile_base[block_s] * 128 + within     # per-core slot index

    nslots = tt * 128
    idx16 = np.full((NCORES, nslots), PADCOL - BIAS, dtype=np.int16)
    rowl_a = np.zeros((NCORES, nslots), dtype=np.float32)
    w0_a = np.zeros((NCORES, nslots), dtype=np.float32)
    w1_a = np.zeros((NCORES, nslots), dtype=np.float32)

    idx16[core_s, slot] = (col_s - BIAS).astype(np.int16)
    rowl_a[core_s, slot] = rl_s
    w0_a[core_s, slot] = w0_s.astype(np.float32)
    w1_a[core_s, slot] = w1_s.astype(np.float32)

    # ensure the LAST slot of every gather chunk has idx >= 0 (dma_gather
    # trims a trailing negative run); swap within the final tile if needed
    nchunks = (NBLOCKS + CHUNK_BLOCKS - 1) // CHUNK_BLOCKS
    for cidx in range(nchunks):
        b1 = min((cidx + 1) * CHUNK_BLOCKS, NBLOCKS)
        end = int(tile_base[b1]) * 128           # one past chunk's last slot
        for k in range(NCORES):
            if idx16[k, end - 1] < 0:
                tile_lo = end - 128
                cand = np.nonzero(idx16[k, tile_lo:end - 1] >= 0)[0]
                assert cand.size > 0, "entire tile has negative idx"
                j = tile_lo + cand[-1]
                for arr in (idx16, rowl_a, w0_a, w1_a):
                    arr[k, j], arr[k, end - 1] = arr[k, end - 1], arr[k, j]

    iota_np = np.tile(np.arange(R, dtype=np.float32), (128, 1))
    in_maps = []
    for k in range(NCORES):
        in_maps.append({
            "hpre": H_pre,
            # idx position q -> partition q%16, free q//16; replicate x8
            "idx": np.ascontiguousarray(
                np.tile(idx16[k].reshape(nslots // 16, 16).T, (8, 1))),
            "rowl": np.ascontiguousarray(rowl_a[k].reshape(tt, 128).T),
            "w0": np.ascontiguousarray(w0_a[k].reshape(tt, 128).T),
            "w1": np.ascontiguousarray(w1_a[k].reshape(tt, 128).T),
            "iota": iota_np,
        })
    return tuple(quotas.tolist()), tt, in_maps


def _make_runner(nc):
    """Build and cache a jitted shard_map executor for the compiled program."""
    import jax
    from jax.sharding import Mesh, PartitionSpec, NamedSharding
    from jax.experimental.shard_map import shard_map
    from concourse import mybir
    from concourse.bass2jax import (_bass_exec_p, partition_id_tensor,
                                    install_neuronx_cc_hook)

    install_neuronx_cc_hook()
    partition_name = nc.partition_id_tensor.name if nc.partition_id_tensor else None
    in_names, out_names, out_avals = [], [], []
    for alloc in nc.m.functions[0].allocations:
        if not isinstance(alloc, mybir.MemoryLocationSet):
            continue
        name = alloc.memorylocations[0].name
        if alloc.kind == "ExternalInput":
            if name != partition_name:
                in_names.append(name)
        elif alloc.kind == "ExternalOutput":
            out_names.append(name)
            out_avals.append(jax.core.ShapedArray(
                tuple(alloc.tensor_shape), mybir.dt.np(alloc.dtype)))
    n_params = len(in_names)
    all_in = in_names + out_names + ([partition_name] if partition_name else [])

    def _body(*args):
        operands = list(args)
        if partition_name is not None:
            operands.append(partition_id_tensor())
        return tuple(_bass_exec_p.bind(
            *operands, out_avals=tuple(out_avals), in_names=tuple(all_in),
            out_names=tuple(out_names), lowering_input_output_aliases=(),
            sim_require_finite=True, sim_require_nnan=True, nc=nc))

    devices = jax.devices()[:NCORES]
    mesh = Mesh(np.asarray(devices), ("core",))
    spec = PartitionSpec("core")
    f = jax.jit(shard_map(_body, mesh=mesh,
                          in_specs=(spec,) * (n_params + len(out_names)),
                          out_specs=(spec,), check_rep=False))
    sharding = NamedSharding(mesh, spec)
    zeros = [np.zeros((av.shape[0] * NCORES,) + av.shape[1:], av.dtype)
             for av in out_avals]
    return {"f": f, "in_names": in_names, "out_names": out_names,
            "sharding": sharding, "zeros": zeros}


def kernel(H_, edge_index, edge_values, weight, num_nodes):
    import jax

    quotas, tt, in_maps = _prepare(H_, edge_index, edge_values, weight)
    key = quotas
    if key not in _prog_cache:
        nc = _build_program(np.array(quotas), tt, nqueues=4, chunk_blocks=10,
                            gbufs=3, selbufs=3, pbufs=3)
        _prog_cache[key] = _make_runner(nc)
    rn = _prog_cache[key]

    args = []
    for name in rn["in_names"]:
        glob = np.concatenate([m[name] for m in in_maps], axis=0)
        args.append(jax.device_put(glob, rn["sharding"]))
    for z in rn["zeros"]:
        args.append(jax.device_put(z, rn["sharding"]))
    outs = rn["f"](*args)
    res = np.asarray(outs[rn["out_names"].index("out_local")])  # [8*128, NBLOCKS*R]

    out = np.empty((C, N, D), dtype=np.float32)
    for k in range(NCORES):
        ol = res[k * 128:(k + 1) * 128]
        out[0, k * RPC:(k + 1) * RPC, :] = ol[0:D, :RPC].T
        out[1, k * RPC:(k + 1) * RPC, :] = ol[D:2 * D, :RPC].T
    return out



# revision 3
# speedup vs baseline: 1.1831x; 1.1831x over previous
"""FastGTLayer GNN message passing on 8 Trainium2 NeuronCores.

Strategy (destination-sharded, gather + selection-matmul scatter, bf16):
- Host: softmax(weight) -> per-edge per-channel weights w_c = filt[c,t]*ev[t,e].
  Edges sharded by destination row range (6250 rows/core), sorted by row,
  grouped into 64-row "blocks"; each block padded to a per-block tile quota
  (max over cores) of 128-edge tiles. H packed bf16 [N, 128] = [c0 feats,
  c1 feats] per node.
- Device (SPMD, one program on 8 cores): for each chunk of 8 blocks,
  dma_gather fetches H rows (256B/edge, bf16) by int16 index with a biased
  base (in_=H_pre[17232:], idx = col-17232 in [-17232,32767]);
  DVE scales the gathered rows in place by w0/w1 (both channels fused into
  the 128-wide feature dim) and builds the one-hot selection matrix
  sel[e, r] = (row==r) once per chunk; PE scatter-adds via one matmul per
  128-edge tile: psum[128cd, 64r] += g'^T @ sel (bf16, full-128-col weights
  -> fast weight load); one ACT eviction per chunk to SBUF; HWDGE writes
  [128,(c,d)] x rows to HBM.
- Host: transpose per-core [128, rows] outputs into [C, N, D].
"""
import sys
if "/opt/trn_rl_repo" not in sys.path:
    sys.path.insert(0, "/opt/trn_rl_repo")

import numpy as np
import ml_dtypes

BF16 = np.dtype(ml_dtypes.bfloat16)

C, T, N, E, D = 2, 4, 50000, 400000, 64
M = T * E
NCORES = 8
RPC = N // NCORES          # 6250 destination rows per core
R = 64                     # rows per block (psum window)
NBLOCKS = (RPC + R - 1) // R   # 98 (last block partially used)
BIAS = N - 32768           # 17232; idx = col - BIAS in [-17232, 32767]
PADCOL = 40000             # pad slots gather this row (positive idx), weight 0
CHUNK_BLOCKS = 4           # blocks per dma_gather instruction

_prog_cache = {}


def _build_program(quotas, tt, nqueues=4, chunk_blocks=None, gbufs=3,
                   selbufs=2, pbufs=2, scratch=16384):
    """Build the SPMD Bass program for per-block tile quotas `quotas` (len
    NBLOCKS, sum tt). Returns compiled Bacc instance."""
    from concourse import bacc, mybir
    import concourse.tile as tile
    from concourse.bass import AP

    nc = bacc.Bacc("TRN2", num_swdge_queues=nqueues, dynamic_dma_scratch_size=scratch)
    hpre = nc.dram_tensor("hpre", [N, 2 * D], mybir.dt.bfloat16, kind="ExternalInput")
    idx = nc.dram_tensor("idx", [128, tt * 8], mybir.dt.int16, kind="ExternalInput")
    rowl = nc.dram_tensor("rowl", [128, tt], mybir.dt.bfloat16, kind="ExternalInput")
    w0 = nc.dram_tensor("w0", [128, tt], mybir.dt.bfloat16, kind="ExternalInput")
    w1 = nc.dram_tensor("w1", [128, tt], mybir.dt.bfloat16, kind="ExternalInput")
    iota = nc.dram_tensor("iota", [128, R], mybir.dt.bfloat16, kind="ExternalInput")
    out_local = nc.dram_tensor("out_local", [128, NBLOCKS * R], mybir.dt.float32,
                               kind="ExternalOutput")

    cb_n = chunk_blocks or CHUNK_BLOCKS
    nchunks = (NBLOCKS + cb_n - 1) // cb_n
    tile_base = np.concatenate([[0], np.cumsum(quotas)]).astype(int)

    with tile.TileContext(nc) as tc:
        with tc.tile_pool(name="meta", bufs=1) as mp, \
             tc.tile_pool(name="gp", bufs=gbufs) as gp, \
             tc.tile_pool(name="selp", bufs=selbufs) as selp, \
             tc.tile_pool(name="stp", bufs=2) as stp, \
             tc.tile_pool(name="pp", bufs=pbufs, space="PSUM") as pp:
            idx_t = mp.tile([128, tt * 8], mybir.dt.int16)
            rowl_t = mp.tile([128, tt], mybir.dt.bfloat16)
            w0_t = mp.tile([128, tt], mybir.dt.bfloat16)
            w1_t = mp.tile([128, tt], mybir.dt.bfloat16)
            iota_t = mp.tile([128, R], mybir.dt.bfloat16)

            nc.gpsimd.dma_start(out=idx_t[:], in_=idx[:])
            nc.gpsimd.dma_start(out=rowl_t[:], in_=rowl[:])
            nc.gpsimd.dma_start(out=w0_t[:], in_=w0[:])
            nc.gpsimd.dma_start(out=w1_t[:], in_=w1[:])
            nc.gpsimd.dma_start(out=iota_t[:], in_=iota[:])

            iota_ap = iota_t[:]

            for c in range(nchunks):
                b0 = c * cb_n
                b1 = min(b0 + cb_n, NBLOCKS)
                tb0, tb1 = tile_base[b0], tile_base[b1]
                ct = int(tb1 - tb0)          # tiles in this chunk
                nidx = ct * 128

                g_t = gp.tile([128, ct, 2 * D], mybir.dt.bfloat16, tag="g")
                nc.gpsimd.dma_gather(
                    g_t[:],
                    hpre[BIAS:, :],
                    idx_t[:, tb0 * 8: tb1 * 8],
                    nidx,
                    nidx,
                    2 * D,
                    queue_num=(c % nqueues),
                    single_packet=False,
                )

                # scale both channel halves in place by the per-edge weights
                nc.vector.tensor_tensor(
                    out=g_t[:, :, 0:D], in0=g_t[:, :, 0:D],
                    in1=w0_t[:, tb0:tb1].to_broadcast([128, ct, D]),
                    op=mybir.AluOpType.mult)
                nc.vector.tensor_tensor(
                    out=g_t[:, :, D:2 * D], in0=g_t[:, :, D:2 * D],
                    in1=w1_t[:, tb0:tb1].to_broadcast([128, ct, D]),
                    op=mybir.AluOpType.mult)

                # one-hot selection for the whole chunk: sel[e, t, r] = (row==r)
                iota_b = AP(iota_ap.tensor, iota_ap.offset,
                            [iota_ap.ap[0], [0, ct], iota_ap.ap[1]])
                sel = selp.tile([128, ct, R], mybir.dt.bfloat16, tag="se")
                nc.vector.tensor_tensor(
                    out=sel[:],
                    in0=rowl_t[:, tb0:tb1].to_broadcast([128, ct, R]),
                    in1=iota_b, op=mybir.AluOpType.is_equal)

                ps = pp.tile([128, (b1 - b0) * R], mybir.dt.float32,
                             space="PSUM", tag="ps")
                for b in range(b0, b1):
                    kb = int(quotas[b])
                    lt0 = int(tile_base[b]) - tb0   # tile index within chunk
                    so = (b - b0) * R
                    for k in range(kb):
                        nc.tensor.matmul(out=ps[:, so:so + R],
                                         lhsT=g_t[:, lt0 + k, :],
                                         rhs=sel[:, lt0 + k, :],
                                         start=(k == 0), stop=(k == kb - 1))

                stage = stp.tile([128, (b1 - b0) * R], mybir.dt.float32, tag="st")
                nc.scalar.copy(out=stage[:], in_=ps[:])
                nc.sync.dma_start(out=out_local[:, b0 * R: b1 * R], in_=stage[:])

    nc.compile()
    return nc


def _prepare(H_, edge_index, edge_values, weight, chunk_blocks=CHUNK_BLOCKS):
    """Host-side preprocessing. Returns (quotas, tt, in_maps)."""
    H_ = np.asarray(H_, dtype=np.float32)
    edge_index = np.asarray(edge_index)
    edge_values = np.asarray(edge_values, dtype=np.float32)
    weight = np.asarray(weight, dtype=np.float64)

    # softmax over edge types per channel
    wexp = np.exp(weight - weight.max(axis=1, keepdims=True))
    filt = (wexp / wexp.sum(axis=1, keepdims=True)).astype(np.float32)  # [C,T]

    row = np.ascontiguousarray(edge_index[:, 0, :]).reshape(-1).astype(np.int64)
    col = np.ascontiguousarray(edge_index[:, 1, :]).reshape(-1).astype(np.int64)
    ev = edge_values.reshape(-1)
    tt_of_edge = np.repeat(np.arange(T), E)
    wc = filt[:, tt_of_edge] * ev[None, :]      # [C, M]

    H_pre = np.ascontiguousarray(
        np.transpose(H_, (1, 0, 2)).reshape(N, C * D).astype(BF16))

    core = row // RPC
    row_local = row - core * RPC
    block = row_local // R                       # [0, NBLOCKS)
    # global sort groups edges by (core, block) since both are row-range based
    perm = np.argsort(row, kind="stable")
    core_s = core[perm]
    block_s = block[perm]
    col_s = col[perm]
    rl_s = (row_local[perm] % R).astype(np.float32)
    w0_s = wc[0][perm]
    w1_s = wc[1][perm]

    cb = core_s * NBLOCKS + block_s
    counts = np.bincount(cb, minlength=NCORES * NBLOCKS).reshape(NCORES, NBLOCKS)
    quotas = np.ceil(counts.max(axis=0) / 128).astype(np.int64)
    quotas = np.maximum(quotas, 1)
    tt = int(quotas.sum())
    tile_base = np.concatenate([[0], np.cumsum(quotas)]).astype(np.int64)

    # slot index for each sorted edge: block b of core k occupies slots
    # [tile_base[b]*128, +counts[k,b]) in core k's slot space
    # offsets within (core, block) groups:
    group_starts = np.concatenate([[0], np.cumsum(counts.reshape(-1))])[:-1]
    within = np.arange(M) - group_starts[cb]     # position within group
    slot = tile_base[block_s] * 128 + within     # per-core slot index

    nslots = tt * 128
    idx16 = np.full((NCORES, nslots), PADCOL - BIAS, dtype=np.int16)
    rowl_a = np.zeros((NCORES, nslots), dtype=np.float32)
    w0_a = np.zeros((NCORES, nslots), dtype=np.float32)
    w1_a = np.zeros((NCORES, nslots), dtype=np.float32)

    idx16[core_s, slot] = (col_s - BIAS).astype(np.int16)
    rowl_a[core_s, slot] = rl_s
    w0_a[core_s, slot] = w0_s.astype(np.float32)
    w1_a[core_s, slot] = w1_s.astype(np.float32)

    # ensure the LAST slot of every gather chunk has idx >= 0 (dma_gather
    # trims a trailing negative run); swap within the final tile if needed
    nchunks = (NBLOCKS + chunk_blocks - 1) // chunk_blocks
    for cidx in range(nchunks):
        b1 = min((cidx + 1) * chunk_blocks, NBLOCKS)
        end = int(tile_base[b1]) * 128           # one past chunk's last slot
        for k in range(NCORES):
            if idx16[k, end - 1] < 0:
                tile_lo = end - 128
                cand = np.nonzero(idx16[k, tile_lo:end - 1] >= 0)[0]
                assert cand.size > 0, "entire tile has negative idx"
                j = tile_lo + cand[-1]
                for arr in (idx16, rowl_a, w0_a, w1_a):
                    arr[k, j], arr[k, end - 1] = arr[k, end - 1], arr[k, j]

    iota_np = np.tile(np.arange(R, dtype=np.float32), (128, 1)).astype(BF16)
    in_maps = []
    for k in range(NCORES):
        in_maps.append({
            "hpre": H_pre,
            # idx position q -> partition q%16, free q//16; replicate x8
            "idx": np.ascontiguousarray(
                np.tile(idx16[k].reshape(nslots // 16, 16).T, (8, 1))),
            "rowl": np.ascontiguousarray(
                rowl_a[k].reshape(tt, 128).T.astype(BF16)),
            "w0": np.ascontiguousarray(w0_a[k].reshape(tt, 128).T.astype(BF16)),
            "w1": np.ascontiguousarray(w1_a[k].reshape(tt, 128).T.astype(BF16)),
            "iota": iota_np,
        })
    return tuple(quotas.tolist()), tt, in_maps


def _make_runner(nc):
    """Build and cache a jitted shard_map executor for the compiled program."""
    import jax
    from jax.sharding import Mesh, PartitionSpec, NamedSharding
    from jax.experimental.shard_map import shard_map
    from concourse import mybir
    from concourse.bass2jax import (_bass_exec_p, partition_id_tensor,
                                    install_neuronx_cc_hook)

    install_neuronx_cc_hook()
    partition_name = nc.partition_id_tensor.name if nc.partition_id_tensor else None
    in_names, out_names, out_avals = [], [], []
    for alloc in nc.m.functions[0].allocations:
        if not isinstance(alloc, mybir.MemoryLocationSet):
            continue
        name = alloc.memorylocations[0].name
        if alloc.kind == "ExternalInput":
            if name != partition_name:
                in_names.append(name)
        elif alloc.kind == "ExternalOutput":
            out_names.append(name)
            out_avals.append(jax.core.ShapedArray(
                tuple(alloc.tensor_shape), mybir.dt.np(alloc.dtype)))
    n_params = len(in_names)
    all_in = in_names + out_names + ([partition_name] if partition_name else [])

    def _body(*args):
        operands = list(args)
        if partition_name is not None:
            operands.append(partition_id_tensor())
        return tuple(_bass_exec_p.bind(
            *operands, out_avals=tuple(out_avals), in_names=tuple(all_in),
            out_names=tuple(out_names), lowering_input_output_aliases=(),
            sim_require_finite=True, sim_require_nnan=True, nc=nc))

    devices = jax.devices()[:NCORES]
    mesh = Mesh(np.asarray(devices), ("core",))
    spec = PartitionSpec("core")
    f = jax.jit(shard_map(_body, mesh=mesh,
                          in_specs=(spec,) * (n_params + len(out_names)),
                          out_specs=(spec,), check_rep=False))
    sharding = NamedSharding(mesh, spec)
    zeros = [np.zeros((av.shape[0] * NCORES,) + av.shape[1:], av.dtype)
             for av in out_avals]
    return {"f": f, "in_names": in_names, "out_names": out_names,
            "sharding": sharding, "zeros": zeros}


def kernel(H_, edge_index, edge_values, weight, num_nodes):
    import jax

    quotas, tt, in_maps = _prepare(H_, edge_index, edge_values, weight)
    key = quotas
    if key not in _prog_cache:
        nc = _build_program(np.array(quotas), tt, nqueues=4,
                            chunk_blocks=CHUNK_BLOCKS, gbufs=3, selbufs=2,
                            pbufs=2)
        _prog_cache[key] = _make_runner(nc)
    rn = _prog_cache[key]

    args = []
    for name in rn["in_names"]:
        glob = np.concatenate([m[name] for m in in_maps], axis=0)
        args.append(jax.device_put(glob, rn["sharding"]))
    for z in rn["zeros"]:
        args.append(jax.device_put(z, rn["sharding"]))
    outs = rn["f"](*args)
    res = np.asarray(outs[rn["out_names"].index("out_local")])  # [8*128, NBLOCKS*R]

    out = np.empty((C, N, D), dtype=np.float32)
    for k in range(NCORES):
        ol = res[k * 128:(k + 1) * 128]
        out[0, k * RPC:(k + 1) * RPC, :] = ol[0:D, :RPC].T
        out[1, k * RPC:(k + 1) * RPC, :] = ol[D:2 * D, :RPC].T
    return out


# revision 5
# speedup vs baseline: 1.6823x; 1.4219x over previous
"""FastGTLayer GNN message passing on 8 Trainium2 NeuronCores.

Strategy (destination-sharded, gather + selection-matmul scatter, bf16):
- Host: softmax(weight) -> per-edge per-channel weights w_c = filt[c,t]*ev[t,e].
  Edges sharded by destination row range (6250 rows/core). Rows are
  bin-packed into 98 bins ("blocks") of <=64 rows each, balancing per-bin
  edge counts across all 8 cores so every block needs ~16 tiles of 128
  edge slots (per-block quota = max over cores). H packed bf16 [N, 128] =
  [c0 feats, c1 feats] per node.
- Device (SPMD, one program on 8 cores): for each chunk of 2 blocks,
  dma_gather fetches H rows (256B/edge, bf16) by int16 index with a biased
  base (in_=H_pre[17232:], idx = col-17232 in [-17232,32767]); the gather
  descriptor rate (~3.5-5ns/edge, SWDGE 4-queue limit) is the kernel
  bottleneck, so chunks are small and deeply pipelined over 4 queues.
  DVE scales the gathered rows in place by w0/w1 (both channels fused into
  the 128-wide feature dim) and builds the one-hot selection matrix
  sel[e, r] = (row==r) once per chunk; PE scatter-adds via one matmul per
  128-edge tile: psum[128cd, 64r] += g'^T @ sel (bf16, full-128-col weights
  -> fast weight load); one ACT eviction per chunk to SBUF; HWDGE writes
  [128,(c,d)] x rows to HBM.
- Host: permutation-unpack per-core [128, rows] outputs into [C, N, D].
"""
import sys
if "/opt/trn_rl_repo" not in sys.path:
    sys.path.insert(0, "/opt/trn_rl_repo")

import heapq
import numpy as np
import ml_dtypes

BF16 = np.dtype(ml_dtypes.bfloat16)

C, T, N, E, D = 2, 4, 50000, 400000, 64
M = T * E
NCORES = 8
RPC = N // NCORES          # 6250 destination rows per core
R = 64                     # rows per block (psum window)
NBLOCKS = (RPC + R - 1) // R   # 98 bins per core
BIAS = N - 32768           # 17232; idx = col - BIAS in [-17232, 32767]
PADCOL = 40000             # pad slots gather this row (positive idx), weight 0
CHUNK_BLOCKS = 2           # blocks per dma_gather instruction

_prog_cache = {}


def _build_program(quotas, tt, nqueues=4, chunk_blocks=None, gbufs=6,
                   selbufs=3, pbufs=2, scratch=16384):
    """Build the SPMD Bass program for per-block tile quotas `quotas` (len
    NBLOCKS, sum tt). Returns compiled Bacc instance."""
    from concourse import bacc, mybir
    import concourse.tile as tile
    from concourse.bass import AP

    nc = bacc.Bacc("TRN2", num_swdge_queues=nqueues, dynamic_dma_scratch_size=scratch)
    hpre = nc.dram_tensor("hpre", [N, 2 * D], mybir.dt.bfloat16, kind="ExternalInput")
    idx = nc.dram_tensor("idx", [128, tt * 8], mybir.dt.int16, kind="ExternalInput")
    rowl = nc.dram_tensor("rowl", [128, tt], mybir.dt.bfloat16, kind="ExternalInput")
    w0 = nc.dram_tensor("w0", [128, tt], mybir.dt.bfloat16, kind="ExternalInput")
    w1 = nc.dram_tensor("w1", [128, tt], mybir.dt.bfloat16, kind="ExternalInput")
    iota = nc.dram_tensor("iota", [128, R], mybir.dt.bfloat16, kind="ExternalInput")
    out_local = nc.dram_tensor("out_local", [128, NBLOCKS * R], mybir.dt.float32,
                               kind="ExternalOutput")

    cb_n = chunk_blocks or CHUNK_BLOCKS
    nchunks = (NBLOCKS + cb_n - 1) // cb_n
    tile_base = np.concatenate([[0], np.cumsum(quotas)]).astype(int)

    with tile.TileContext(nc) as tc:
        with tc.tile_pool(name="meta", bufs=1) as mp, \
             tc.tile_pool(name="gp", bufs=gbufs) as gp, \
             tc.tile_pool(name="selp", bufs=selbufs) as selp, \
             tc.tile_pool(name="stp", bufs=2) as stp, \
             tc.tile_pool(name="pp", bufs=pbufs, space="PSUM") as pp:
            idx_t = mp.tile([128, tt * 8], mybir.dt.int16)
            rowl_t = mp.tile([128, tt], mybir.dt.bfloat16)
            w0_t = mp.tile([128, tt], mybir.dt.bfloat16)
            w1_t = mp.tile([128, tt], mybir.dt.bfloat16)
            iota_t = mp.tile([128, R], mybir.dt.bfloat16)

            nc.gpsimd.dma_start(out=idx_t[:], in_=idx[:])
            nc.gpsimd.dma_start(out=rowl_t[:], in_=rowl[:])
            nc.gpsimd.dma_start(out=w0_t[:], in_=w0[:])
            nc.gpsimd.dma_start(out=w1_t[:], in_=w1[:])
            nc.gpsimd.dma_start(out=iota_t[:], in_=iota[:])

            iota_ap = iota_t[:]

            for c in range(nchunks):
                b0 = c * cb_n
                b1 = min(b0 + cb_n, NBLOCKS)
                tb0, tb1 = tile_base[b0], tile_base[b1]
                ct = int(tb1 - tb0)          # tiles in this chunk
                nidx = ct * 128

                g_t = gp.tile([128, ct, 2 * D], mybir.dt.bfloat16, tag="g")
                nc.gpsimd.dma_gather(
                    g_t[:],
                    hpre[BIAS:, :],
                    idx_t[:, tb0 * 8: tb1 * 8],
                    nidx,
                    nidx,
                    2 * D,
                    queue_num=(c % nqueues),
                    single_packet=False,
                )

                # scale both channel halves in place by the per-edge weights
                nc.vector.tensor_tensor(
                    out=g_t[:, :, 0:D], in0=g_t[:, :, 0:D],
                    in1=w0_t[:, tb0:tb1].to_broadcast([128, ct, D]),
                    op=mybir.AluOpType.mult)
                nc.vector.tensor_tensor(
                    out=g_t[:, :, D:2 * D], in0=g_t[:, :, D:2 * D],
                    in1=w1_t[:, tb0:tb1].to_broadcast([128, ct, D]),
                    op=mybir.AluOpType.mult)

                # one-hot selection for the whole chunk: sel[e, t, r] = (row==r)
                iota_b = AP(iota_ap.tensor, iota_ap.offset,
                            [iota_ap.ap[0], [0, ct], iota_ap.ap[1]])
                sel = selp.tile([128, ct, R], mybir.dt.bfloat16, tag="se")
                nc.vector.tensor_tensor(
                    out=sel[:],
                    in0=rowl_t[:, tb0:tb1].to_broadcast([128, ct, R]),
                    in1=iota_b, op=mybir.AluOpType.is_equal)

                ps = pp.tile([128, (b1 - b0) * R], mybir.dt.float32,
                             space="PSUM", tag="ps")
                for b in range(b0, b1):
                    kb = int(quotas[b])
                    lt0 = int(tile_base[b]) - tb0   # tile index within chunk
                    so = (b - b0) * R
                    for k in range(kb):
                        nc.tensor.matmul(out=ps[:, so:so + R],
                                         lhsT=g_t[:, lt0 + k, :],
                                         rhs=sel[:, lt0 + k, :],
                                         start=(k == 0), stop=(k == kb - 1))

                stage = stp.tile([128, (b1 - b0) * R], mybir.dt.float32, tag="st")
                nc.scalar.copy(out=stage[:], in_=ps[:])
                nc.sync.dma_start(out=out_local[:, b0 * R: b1 * R], in_=stage[:])

    nc.compile()
    return nc


def _binpack_rows(counts_row):
    """Assign RPC rows to NBLOCKS bins (<=R rows each), balancing edge sums
    (serpentine deal by descending count + pairwise swap repair). Returns
    (block_of_row [RPC], pos_of_row [RPC], rowlist [NBLOCKS, R], -1 pad)."""
    order = np.argsort(-counts_row, kind="stable")
    binrows = [[] for _ in range(NBLOCKS)]
    sums = np.zeros(NBLOCKS, dtype=np.int64)
    i = 0
    fwd = True
    while i < RPC:
        seq = range(NBLOCKS) if fwd else range(NBLOCKS - 1, -1, -1)
        for b in seq:
            if i >= RPC:
                break
            if len(binrows[b]) >= R:
                continue
            r = order[i]
            i += 1
            binrows[b].append(r)
            sums[b] += counts_row[r]
        fwd = not fwd
    for _ in range(4000):
        bmax = int(np.argmax(sums))
        bmin = int(np.argmin(sums))
        gap = sums[bmax] - sums[bmin]
        if gap <= 1:
            break
        ra = np.array(binrows[bmax])
        rb = np.array(binrows[bmin])
        d = counts_row[ra][:, None] - counts_row[rb][None, :]
        ji = np.unravel_index(np.argmin(np.abs(d - gap / 2)), d.shape)
        delta = d[ji]
        if delta <= 0:
            break
        a_r, b_r = ra[ji[0]], rb[ji[1]]
        binrows[bmax][ji[0]] = b_r
        binrows[bmin][ji[1]] = a_r
        sums[bmax] -= delta
        sums[bmin] += delta

    block_of_row = np.empty(RPC, dtype=np.int64)
    pos_of_row = np.empty(RPC, dtype=np.int64)
    rowlist = np.full((NBLOCKS, R), -1, dtype=np.int64)
    for b in range(NBLOCKS):
        for j, r in enumerate(binrows[b]):
            block_of_row[r] = b
            pos_of_row[r] = j
            rowlist[b, j] = r
    return block_of_row, pos_of_row, rowlist


def _prepare(H_, edge_index, edge_values, weight, chunk_blocks=CHUNK_BLOCKS):
    """Host-side preprocessing. Returns (quotas, tt, in_maps, rowmaps)."""
    H_ = np.asarray(H_, dtype=np.float32)
    edge_index = np.asarray(edge_index)
    edge_values = np.asarray(edge_values, dtype=np.float32)
    weight = np.asarray(weight, dtype=np.float64)

    # softmax over edge types per channel
    wexp = np.exp(weight - weight.max(axis=1, keepdims=True))
    filt = (wexp / wexp.sum(axis=1, keepdims=True)).astype(np.float32)  # [C,T]

    row = np.ascontiguousarray(edge_index[:, 0, :]).reshape(-1).astype(np.int64)
    col = np.ascontiguousarray(edge_index[:, 1, :]).reshape(-1).astype(np.int64)
    ev = edge_values.reshape(-1)
    tt_of_edge = np.repeat(np.arange(T), E)
    wc = filt[:, tt_of_edge] * ev[None, :]      # [C, M]

    H_pre = np.ascontiguousarray(
        np.transpose(H_, (1, 0, 2)).reshape(N, C * D).astype(BF16))

    core = row // RPC
    row_local = row - core * RPC

    # per-core bin packing of rows into blocks (balances per-block edges)
    block_of = np.empty((NCORES, RPC), dtype=np.int64)
    pos_of = np.empty((NCORES, RPC), dtype=np.int64)
    rowmaps = np.empty((NCORES, NBLOCKS, R), dtype=np.int64)
    for k in range(NCORES):
        counts_row = np.bincount(row_local[core == k], minlength=RPC)
        b_of, p_of, rl = _binpack_rows(counts_row)
        block_of[k] = b_of
        pos_of[k] = p_of
        rowmaps[k] = rl

    block = block_of[core, row_local]            # [M]
    rl_pos = pos_of[core, row_local]             # [M] position within block

    # sort edges by (core, block)
    cb = core * NBLOCKS + block
    perm = np.argsort(cb, kind="stable")
    cb_s = cb[perm]
    core_s = core[perm]
    block_s = block[perm]
    col_s = col[perm]
    rl_s = rl_pos[perm].astype(np.float32)
    w0_s = wc[0][perm]
    w1_s = wc[1][perm]

    counts = np.bincount(cb, minlength=NCORES * NBLOCKS).reshape(NCORES, NBLOCKS)
    quotas = np.ceil(counts.max(axis=0) / 128).astype(np.int64)
    quotas = np.maximum(quotas, 1)
    tt = int(quotas.sum())
    tile_base = np.concatenate([[0], np.cumsum(quotas)]).astype(np.int64)

    group_starts = np.concatenate([[0], np.cumsum(counts.reshape(-1))])[:-1]
    within = np.arange(M) - group_starts[cb_s]   # position within group
    slot = tile_base[block_s] * 128 + within     # per-core slot index

    nslots = tt * 128
    idx16 = np.full((NCORES, nslots), PADCOL - BIAS, dtype=np.int16)
    rowl_a = np.zeros((NCORES, nslots), dtype=np.float32)
    w0_a = np.zeros((NCORES, nslots), dtype=np.float32)
    w1_a = np.zeros((NCORES, nslots), dtype=np.float32)

    idx16[core_s, slot] = (col_s - BIAS).astype(np.int16)
    rowl_a[core_s, slot] = rl_s
    w0_a[core_s, slot] = w0_s.astype(np.float32)
    w1_a[core_s, slot] = w1_s.astype(np.float32)

    # ensure the LAST slot of every gather chunk has idx >= 0 (dma_gather
    # trims a trailing negative run); swap within the final tile if needed
    nchunks = (NBLOCKS + chunk_blocks - 1) // chunk_blocks
    for cidx in range(nchunks):
        b1 = min((cidx + 1) * chunk_blocks, NBLOCKS)
        end = int(tile_base[b1]) * 128           # one past chunk's last slot
        for k in range(NCORES):
            if idx16[k, end - 1] < 0:
                tile_lo = end - 128
                cand = np.nonzero(idx16[k, tile_lo:end - 1] >= 0)[0]
                assert cand.size > 0, "entire tile has negative idx"
                j = tile_lo + cand[-1]
                for arr in (idx16, rowl_a, w0_a, w1_a):
                    arr[k, j], arr[k, end - 1] = arr[k, end - 1], arr[k, j]

    iota_np = np.tile(np.arange(R, dtype=np.float32), (128, 1)).astype(BF16)
    in_maps = []
    for k in range(NCORES):
        in_maps.append({
            "hpre": H_pre,
            # idx position q -> partition q%16, free q//16; replicate x8
            "idx": np.ascontiguousarray(
                np.tile(idx16[k].reshape(nslots // 16, 16).T, (8, 1))),
            "rowl": np.ascontiguousarray(
                rowl_a[k].reshape(tt, 128).T.astype(BF16)),
            "w0": np.ascontiguousarray(w0_a[k].reshape(tt, 128).T.astype(BF16)),
            "w1": np.ascontiguousarray(w1_a[k].reshape(tt, 128).T.astype(BF16)),
            "iota": iota_np,
        })
    return tuple(quotas.tolist()), tt, in_maps, rowmaps


def _make_runner(nc):
    """Build and cache a jitted shard_map executor for the compiled program."""
    import jax
    from jax.sharding import Mesh, PartitionSpec, NamedSharding
    from jax.experimental.shard_map import shard_map
    from concourse import mybir
    from concourse.bass2jax import (_bass_exec_p, partition_id_tensor,
                                    install_neuronx_cc_hook)

    install_neuronx_cc_hook()
    partition_name = nc.partition_id_tensor.name if nc.partition_id_tensor else None
    in_names, out_names, out_avals = [], [], []
    for alloc in nc.m.functions[0].allocations:
        if not isinstance(alloc, mybir.MemoryLocationSet):
            continue
        name = alloc.memorylocations[0].name
        if alloc.kind == "ExternalInput":
            if name != partition_name:
                in_names.append(name)
        elif alloc.kind == "ExternalOutput":
            out_names.append(name)
            out_avals.append(jax.core.ShapedArray(
                tuple(alloc.tensor_shape), mybir.dt.np(alloc.dtype)))
    n_params = len(in_names)
    all_in = in_names + out_names + ([partition_name] if partition_name else [])

    def _body(*args):
        operands = list(args)
        if partition_name is not None:
            operands.append(partition_id_tensor())
        return tuple(_bass_exec_p.bind(
            *operands, out_avals=tuple(out_avals), in_names=tuple(all_in),
            out_names=tuple(out_names), lowering_input_output_aliases=(),
            sim_require_finite=True, sim_require_nnan=True, nc=nc))

    devices = jax.devices()[:NCORES]
    mesh = Mesh(np.asarray(devices), ("core",))
    spec = PartitionSpec("core")
    f = jax.jit(shard_map(_body, mesh=mesh,
                          in_specs=(spec,) * (n_params + len(out_names)),
                          out_specs=(spec,), check_rep=False))
    sharding = NamedSharding(mesh, spec)
    zeros = [np.zeros((av.shape[0] * NCORES,) + av.shape[1:], av.dtype)
             for av in out_avals]
    return {"f": f, "in_names": in_names, "out_names": out_names,
            "sharding": sharding, "zeros": zeros}


def kernel(H_, edge_index, edge_values, weight, num_nodes):
    import jax

    quotas, tt, in_maps, rowmaps = _prepare(H_, edge_index, edge_values, weight)
    key = quotas
    if key not in _prog_cache:
        nc = _build_program(np.array(quotas), tt, nqueues=4,
                            chunk_blocks=CHUNK_BLOCKS, gbufs=6, selbufs=3,
                            pbufs=2)
        _prog_cache[key] = _make_runner(nc)
    rn = _prog_cache[key]

    args = []
    for name in rn["in_names"]:
        glob = np.concatenate([m[name] for m in in_maps], axis=0)
        args.append(jax.device_put(glob, rn["sharding"]))
    for z in rn["zeros"]:
        args.append(jax.device_put(z, rn["sharding"]))
    outs = rn["f"](*args)
    res = np.asarray(outs[rn["out_names"].index("out_local")])  # [8*128, NBLOCKS*R]

    out = np.empty((C, N, D), dtype=np.float32)
    for k in range(NCORES):
        ol = res[k * 128:(k + 1) * 128]          # [128, NBLOCKS*R]
        rm = rowmaps[k].reshape(-1)              # [NBLOCKS*R], -1 = unused
        valid = rm >= 0
        gr = k * RPC + rm[valid]                 # global rows
        out[0, gr, :] = ol[0:D, valid].T
        out[1, gr, :] = ol[D:2 * D, valid].T
    return out
